# revision 19
# baseline (speedup 1.0000x reference)
"""Trainium2 Bass kernel for nn_Attention_75402445849133.

Dense per-batch attention:
  q = Wq @ x[b] + bq ; k = Wk @ x[b] + bk ; v = x[b] (unprojected)
  per head h (16 heads, d=64, S=128):
    scores = (q_h^T k_h) / 8 ; attn = softmax(scores) ; out_h = attn @ v_h^T
  score[b, f] = sum_s out[f, s] * Wo[s] + bo

Sharded data-parallel over batch B=256 across 8 NeuronCores (32 b/core).
All matmul operands fp16 (fp32 PSUM accumulation).

Key tricks:
  - scores computed TRANSPOSED (t on partitions) so softmax denominator and
    the AV matmul both contract over t on partitions with no attn transpose.
  - x[b]^T built once per b via 8 PE transposes; stored with a constant
    ones column appended after each head's 64 columns, so the AV matmul
    (stationary = exp(scores)) also emits the softmax denominator column.
  - softmax division + Wo projection weight folded into a per-partition
    scale on the PSUM->SBUF copy (ACT engine, scale=Wo[s]/colsum[s]).
  - final f-projection = 2 matmuls with a ones stationary vector over the
    concatenated scaled head outputs (128 x 1024).
  - bo added on host.
"""

import sys
import types

import numpy as np

from concourse import bass, bacc, tile, mybir
from concourse.bass_utils import run_bass_kernel_spmd


def _ensure_axon_hooks():
    """Provide antenv.axon_hooks if the image lacks it (needed for trace=True)."""
    try:
        import antenv.axon_hooks  # noqa: F401

        return
    except ImportError:
        pass
    import antenv

    mod = types.ModuleType("antenv.axon_hooks")
    mod._hook = None
    mod.set_axon_ntff_profile_hook = lambda h: setattr(mod, "_hook", h)
    mod.get_axon_ntff_profile_hook = lambda: mod._hook
    sys.modules["antenv.axon_hooks"] = mod
    antenv.axon_hooks = mod
    try:
        from trn_agent_boot.trn_boot import _ntff_profile_via_ctypes

        hook = _ntff_profile_via_ctypes("/opt/axon/libaxon_pjrt.so")
        if hook is not None:
            mod._hook = hook
    except Exception:
        pass


_ensure_axon_hooks()

F16 = mybir.dt.float16
F32 = mybir.dt.float32

N_CORES = 8
B = 256
F_IN = 1024
HID = 1024
H = 16
S = 128
D = 64  # head dim (both q/k and v)
KT = 8  # k tiles (F_IN / 128)
MT = 8  # m tiles (HID / 128)
TEMP = 8.0

TRACE = False  # test.py sets this for profiling runs


def build_bass(n_groups=8, G=4):
    """Build the per-core Bass graph. NB = n_groups * G local batches."""
    NB = n_groups * G
    NQK = G * S  # moving free dim of the QK matmuls

    nc = bacc.Bacc(None, target_bir_lowering=False)

    # host-prepared inputs (per core)
    xr = nc.dram_tensor("xr", [n_groups, 128, KT, G, S], F16, kind="ExternalInput")
    # x[b]^T per batch with a ones column after each head's 64 cols (baked on host)
    xtr = nc.dram_tensor("xtr", [n_groups, 128, G, H, D + 1], F16, kind="ExternalInput")
    wqt = nc.dram_tensor("wqt", [MT, 128, KT, 128], F16, kind="ExternalInput")
    wkt = nc.dram_tensor("wkt", [MT, 128, KT, 128], F16, kind="ExternalInput")
    bqr = nc.dram_tensor("bqr", [128, MT], F32, kind="ExternalInput")
    bkr = nc.dram_tensor("bkr", [128, MT], F32, kind="ExternalInput")
    wo16 = nc.dram_tensor("wo16", [128, 1], F16, kind="ExternalInput")
    out = nc.dram_tensor("out", [NB, F_IN], F32, kind="ExternalOutput")

    with tile.TileContext(nc) as tc:
        with (
            tc.tile_pool(name="consts", bufs=1) as cpool,
            tc.tile_pool(name="xp", bufs=2) as xpool,
            tc.tile_pool(name="xtp", bufs=3) as xtpool,
            tc.tile_pool(name="qkp", bufs=2) as qkpool,
            tc.tile_pool(name="ep", bufs=4) as epool,
            tc.tile_pool(name="wfp", bufs=4) as wfpool,
            tc.tile_pool(name="uop", bufs=2) as uopool,
            tc.tile_pool(name="orow", bufs=2) as orowpool,
            tc.tile_pool(name="ps_qk", bufs=2, space="PSUM") as ps_qk,
            tc.tile_pool(name="ps_sc", bufs=2, space="PSUM") as ps_sc,
            tc.tile_pool(name="ps_uo", bufs=2, space="PSUM") as ps_uo,
        ):
            # ---- persistent tiles ----
            # per-mt weight tiles so the first matmul only waits on one DMA
            wq_ts = [
                cpool.tile([128, KT, 128], F16, name=f"wq{mt}", tag=f"wq{mt}")
                for mt in range(MT)
            ]
            wk_ts = [
                cpool.tile([128, KT, 128], F16, name=f"wk{mt}", tag=f"wk{mt}")
                for mt in range(MT)
            ]
            bq_t = cpool.tile([128, MT], F32, tag="bq")
            bk_t = cpool.tile([128, MT], F32, tag="bk")
            wo_t = cpool.tile([128, 1], F16, tag="wo")
            zero_t = cpool.tile([128, 1], F32, tag="zero")
            warm_t = cpool.tile([128, 512], F16, tag="warm")

            nc.vector.memset(zero_t[:], 0.0)
            nc.vector.memset(warm_t[:], 0.0)

            # first x group + interleaved per-mt weight DMAs, so the PE can
            # start as soon as x(g0) + Wq(mt0) have landed
            x16_first = xpool.tile([128, KT, G, S], F16, tag="x16")
            nc.sync.dma_start(
                x16_first[:], xr[0].rearrange("p kt g s -> p (kt g s)")
            )
            for mt in range(MT):
                nc.sync.dma_start(
                    wq_ts[mt][:], wqt[mt].rearrange("p kt m -> p (kt m)")
                )
                nc.sync.dma_start(
                    wk_ts[mt][:], wkt[mt].rearrange("p kt m -> p (kt m)")
                )
            nc.sync.dma_start(bq_t[:], bqr[:])
            nc.sync.dma_start(bk_t[:], bkr[:])
            nc.sync.dma_start(wo_t[:], wo16[:])

            # PE warm-up: dummy matmuls during the initial DMA wait keep the
            # HAM activity monitor busy so real matmuls run at full clock
            warm_ps = ps_qk.tile([128, 512], F32, tag="qk")
            for _ in range(20):
                nc.tensor.matmul(warm_ps[:], warm_t[:, 0:128], warm_t[:])

            pending = []

            def finalize(item):
                # final projection: score[f] = sum_s Wo[s] * uo_sc[s, f]
                fb, uo = item
                ps_f = ps_qk.tile([33, 512], F32, tag="qk")
                nc.tensor.matmul(ps_f[0:1, :], wo_t[:], uo[:, 0:512])
                nc.tensor.matmul(ps_f[32:33, :], wo_t[:], uo[:, 512:1024])
                orow = orowpool.tile([1, F_IN], F32, tag="orow")
                nc.scalar.copy(orow[0:1, 0:512], ps_f[0:1, :])
                nc.scalar.copy(orow[0:1, 512:1024], ps_f[32:33, :])
                nc.sync.dma_start(out[fb : fb + 1, :], orow[:])

            for grp in range(n_groups):
                # ---- load x group: (128, KT, G, S) fp16 ----
                if grp == 0:
                    x16 = x16_first
                else:
                    x16 = xpool.tile([128, KT, G, S], F16, tag="x16")
                    nc.sync.dma_start(
                        x16[:], xr[grp].rearrange("p kt g s -> p (kt g s)")
                    )

                # ---- QK projections: q/k = W @ x (+bias), fp16 out ----
                q_sb = qkpool.tile([128, MT, NQK], F16, tag="q")
                k_sb = qkpool.tile([128, MT, NQK], F16, tag="k")
                for mt in range(MT):
                    for w_ts, b_t, dst in (
                        (wq_ts, bq_t, q_sb),
                        (wk_ts, bk_t, k_sb),
                    ):
                        ps = ps_qk.tile([128, NQK], F32, tag="qk")
                        for kt in range(KT):
                            nc.tensor.matmul(
                                ps[:],
                                w_ts[mt][:, kt, :],
                                x16[:, kt, :, :].rearrange("p g s -> p (g s)"),
                                start=(kt == 0),
                                stop=(kt == KT - 1),
                            )
                        # bias add + fp16 cast (DVE)
                        nc.vector.tensor_scalar_add(
                            dst[:, mt, :], ps[:], b_t[:, mt : mt + 1]
                        )

                # ---- attention per local batch ----
                for g in range(G):
                    b_loc = grp * G + g

                    # x[b]^T (with baked ones columns) straight from DRAM
                    xT = xtpool.tile([128, H, D + 1], F16, tag="xT")
                    nc.sync.dma_start(
                        xT[:], xtr[grp, :, g].rearrange("p h d -> p (h d)")
                    )

                    uo_sc = uopool.tile([128, H * D], F16, tag="uosc")
                    uo_view = uo_sc[:].rearrange(
                        "p (pair par d) -> p pair par d", par=2, d=D
                    )
                    # Octets: even heads -> cols 0-511 (bank A, PE rows 0-63),
                    # odd heads -> cols 512-1023 (bank B, rows 64-127).
                    # Matmuls into one PSUM bank must share a PE row group
                    # (mixing is concurrent and fatal), but across banks the
                    # two row groups run concurrently.
                    for oc in range(2):
                        ps_s = ps_sc.tile([128, 2, 4 * S], F32, tag="sc")
                        for j in range(4):
                            mt = oc * 4 + j
                            for par in (0, 1):
                                po = par * D
                                # scoresT[t, s] = sum_d k[d,t] * q[d,s]
                                nc.tensor.matmul(
                                    ps_s[:, par, j * S : (j + 1) * S],
                                    k_sb[po : po + D, mt, g * S : (g + 1) * S],
                                    q_sb[po : po + D, mt, g * S : (g + 1) * S],
                                )
                        for par in (0, 1):
                            heads = [2 * (oc * 4 + j) + par for j in range(4)]
                            # one batched exp per parity quad (ACT)
                            E = epool.tile([128, 4 * S], F16, tag="E")
                            nc.scalar.activation(
                                E[:],
                                ps_s[:, par, :],
                                mybir.ActivationFunctionType.Exp,
                                bias=zero_t[:, 0:1],
                                scale=1.0 / TEMP,
                            )
                            ps_u = ps_uo.tile([128, 4, D + 1], F32, tag="uo")
                            for hi, h in enumerate(heads):
                                # uoutT[s, 0:64] = sum_t E[t,s] * xT[t, d]
                                # uoutT[s, 64]   = sum_t E[t,s]  (ones col)
                                nc.tensor.matmul(
                                    ps_u[:, hi, :],
                                    E[:, hi * S : (hi + 1) * S],
                                    xT[:, h, :],
                                )
                            # rc[s, hi] = 1 / colsum_hi[s]
                            rc = wfpool.tile([128, 4], F32, tag="rc")
                            nc.vector.reciprocal(rc[:], ps_u[:, :, D])
                            # uo_sc[s, (h,d)] = uoutT[s, (h,d)] * rc[s, h]
                            nc.vector.tensor_mul(
                                uo_view[:, oc * 4 : (oc + 1) * 4, par, :],
                                ps_u[:, :, 0:D],
                                rc[:].unsqueeze(2).broadcast_to((128, 4, D)),
                            )

                    # finals are deferred one batch so the PE never waits on
                    # the DVE normalization chain (software pipelining)
                    pending.append((b_loc, uo_sc))
                    if len(pending) > 1:
                        finalize(pending.pop(0))

            while pending:
                finalize(pending.pop(0))

    nc.compile()
    return nc


def prep_inputs(x, Wq, bq, Wk, bk, Wo, n_groups=8, G=4, n_cores=N_CORES):
    """Host-side shard + layout prep. Returns in_maps for run_bass_kernel_spmd."""
    x = np.asarray(x, dtype=np.float32)
    nb = n_groups * G
    x16 = x.astype(np.float16)
    # (c, grp, g, kt, p, s) -> (c, grp, p, kt, g, s)
    xr = (
        x16.reshape(n_cores, n_groups, G, KT, 128, S)
        .transpose(0, 1, 4, 3, 2, 5)
        .copy()
    )
    # x^T per batch with ones col per head: (c, grp, t, g, h, 65)
    xtr = np.ones((n_cores, n_groups, S, G, H, D + 1), dtype=np.float16)
    xtr[..., 0:D] = x16.reshape(n_cores, n_groups, G, H, D, S).transpose(
        0, 1, 5, 2, 3, 4
    )
    # W.T is (k, m); lay out as (mt, p, kt, 128) so each mt tile is one DMA
    wqt = np.ascontiguousarray(
        np.asarray(Wq, dtype=np.float32).T.reshape(KT, 128, MT, 128).transpose(2, 1, 0, 3)
    ).astype(np.float16)
    wkt = np.ascontiguousarray(
        np.asarray(Wk, dtype=np.float32).T.reshape(KT, 128, MT, 128).transpose(2, 1, 0, 3)
    ).astype(np.float16)
    bqr = np.ascontiguousarray(np.asarray(bq, dtype=np.float32).reshape(MT, 128).T)
    bkr = np.ascontiguousarray(np.asarray(bk, dtype=np.float32).reshape(MT, 128).T)
    wo_a = np.asarray(Wo, dtype=np.float32).reshape(128, 1).astype(np.float16)
    in_maps = []
    for c in range(n_cores):
        in_maps.append(
            {
                "xr": xr[c],
                "xtr": xtr[c],
                "wqt": wqt,
                "wkt": wkt,
                "bqr": bqr,
                "bkr": bkr,
                "wo16": wo_a,
            }
        )
    return in_maps


_NC_CACHE = {}


def kernel(x, Wq, bq, Wk, bk, Wo, bo):
    key = "full"
    if key not in _NC_CACHE:
        _NC_CACHE[key] = build_bass()
    nc = _NC_CACHE[key]

    in_maps = prep_inputs(x, Wq, bq, Wk, bk, Wo)
    res = run_bass_kernel_spmd(nc, in_maps, core_ids=list(range(N_CORES)), trace=TRACE)
    kernel.last_result = res
    out = np.concatenate([res.results[c]["out"] for c in range(N_CORES)], axis=0)
    out = out + np.float32(np.asarray(bo).reshape(-1)[0])
    return out.astype(np.float32)


# revision 23
# speedup vs baseline: 1.1035x; 1.1035x over previous
"""Trainium2 Bass kernel for nn_Attention_75402445849133.

Dense per-batch attention:
  q = Wq @ x[b] + bq ; k = Wk @ x[b] + bk ; v = x[b] (unprojected)
  per head h (16 heads, d=64, S=128):
    scores = (q_h^T k_h) / 8 ; attn = softmax(scores) ; out_h = attn @ v_h^T
  score[b, f] = sum_s out[f, s] * Wo[s] + bo

Sharded data-parallel over batch B=256 across 8 NeuronCores (32 b/core).
All matmul operands fp16 (fp32 PSUM accumulation).

Key tricks:
  - scores computed TRANSPOSED (t on partitions) so softmax denominator and
    the AV matmul both contract over t on partitions with no attn transpose.
  - x[b]^T built once per b via 8 PE transposes; stored with a constant
    ones column appended after each head's 64 columns, so the AV matmul
    (stationary = exp(scores)) also emits the softmax denominator column.
  - softmax division + Wo projection weight folded into a per-partition
    scale on the PSUM->SBUF copy (ACT engine, scale=Wo[s]/colsum[s]).
  - final f-projection = 2 matmuls with a ones stationary vector over the
    concatenated scaled head outputs (128 x 1024).
  - bo added on host.
"""

import sys
import types

import numpy as np

from concourse import bass, bacc, tile, mybir
from concourse.bass_utils import run_bass_kernel_spmd


def _ensure_axon_hooks():
    """Provide antenv.axon_hooks if the image lacks it (needed for trace=True)."""
    try:
        import antenv.axon_hooks  # noqa: F401

        return
    except ImportError:
        pass
    import antenv

    mod = types.ModuleType("antenv.axon_hooks")
    mod._hook = None
    mod.set_axon_ntff_profile_hook = lambda h: setattr(mod, "_hook", h)
    mod.get_axon_ntff_profile_hook = lambda: mod._hook
    sys.modules["antenv.axon_hooks"] = mod
    antenv.axon_hooks = mod
    try:
        from trn_agent_boot.trn_boot import _ntff_profile_via_ctypes

        hook = _ntff_profile_via_ctypes("/opt/axon/libaxon_pjrt.so")
        if hook is not None:
            mod._hook = hook
    except Exception:
        pass


_ensure_axon_hooks()

F16 = mybir.dt.float16
F32 = mybir.dt.float32

N_CORES = 8
B = 256
F_IN = 1024
HID = 1024
H = 16
S = 128
D = 64  # head dim (both q/k and v)
KT = 8  # k tiles (F_IN / 128)
MT = 8  # m tiles (HID / 128)
TEMP = 8.0

TRACE = False  # test.py sets this for profiling runs


def build_bass(n_groups=8, G=4):
    """Build the per-core Bass graph. NB = n_groups * G local batches."""
    NB = n_groups * G
    NQK = G * S  # moving free dim of the QK matmuls

    nc = bacc.Bacc(None, target_bir_lowering=False)

    # host-prepared inputs (per core)
    xr = nc.dram_tensor("xr", [n_groups, 128, KT, G, S], F16, kind="ExternalInput")
    # x[b]^T per batch with a ones column after each head's 64 cols (baked on host)
    xtr = nc.dram_tensor("xtr", [n_groups, 128, G, H, D + 1], F16, kind="ExternalInput")
    wqt = nc.dram_tensor("wqt", [MT, 128, KT, 128], F16, kind="ExternalInput")
    wkt = nc.dram_tensor("wkt", [MT, 128, KT, 128], F16, kind="ExternalInput")
    bqr = nc.dram_tensor("bqr", [128, MT], F32, kind="ExternalInput")
    bkr = nc.dram_tensor("bkr", [128, MT], F32, kind="ExternalInput")
    wo16 = nc.dram_tensor("wo16", [128, 1], F16, kind="ExternalInput")
    out = nc.dram_tensor("out", [NB, F_IN], F32, kind="ExternalOutput")

    with tile.TileContext(nc) as tc:
        with (
            tc.tile_pool(name="consts", bufs=1) as cpool,
            tc.tile_pool(name="xp", bufs=2) as xpool,
            tc.tile_pool(name="xtp", bufs=3) as xtpool,
            tc.tile_pool(name="qkp", bufs=2) as qkpool,
            tc.tile_pool(name="ep", bufs=4) as epool,
            tc.tile_pool(name="wfp", bufs=4) as wfpool,
            tc.tile_pool(name="uop", bufs=2) as uopool,
            tc.tile_pool(name="orow", bufs=2) as orowpool,
            tc.tile_pool(name="ps_qk", bufs=2, space="PSUM") as ps_qk,
            tc.tile_pool(name="ps_sc", bufs=3, space="PSUM") as ps_sc,
            tc.tile_pool(name="ps_uo", bufs=2, space="PSUM") as ps_uo,
            tc.tile_pool(name="ps_fi", bufs=1, space="PSUM") as ps_fi,
        ):
            # ---- persistent tiles ----
            # per-mt weight tiles so the first matmul only waits on one DMA
            wq_ts = [
                cpool.tile([128, KT, 128], F16, name=f"wq{mt}", tag=f"wq{mt}")
                for mt in range(MT)
            ]
            wk_ts = [
                cpool.tile([128, KT, 128], F16, name=f"wk{mt}", tag=f"wk{mt}")
                for mt in range(MT)
            ]
            bq_t = cpool.tile([128, MT], F32, tag="bq")
            bk_t = cpool.tile([128, MT], F32, tag="bk")
            wo_t = cpool.tile([128, 1], F16, tag="wo")
            zero_t = cpool.tile([128, 1], F32, tag="zero")

            nc.vector.memset(zero_t[:], 0.0)

            # first x group + interleaved per-mt weight DMAs, so the PE can
            # start as soon as x(g0) + Wq(mt0) have landed
            x16_first = xpool.tile([128, KT, G, S], F16, tag="x16")
            nc.sync.dma_start(
                x16_first[:], xr[0].rearrange("p kt g s -> p (kt g s)")
            )
            for mt in range(MT):
                nc.sync.dma_start(
                    wq_ts[mt][:], wqt[mt].rearrange("p kt m -> p (kt m)")
                )
                nc.sync.dma_start(
                    wk_ts[mt][:], wkt[mt].rearrange("p kt m -> p (kt m)")
                )
            nc.sync.dma_start(bq_t[:], bqr[:])
            nc.sync.dma_start(bk_t[:], bkr[:])
            nc.sync.dma_start(wo_t[:], wo16[:])

            pending = []

            def finalize(item):
                # final projection: score[f] = sum_s Wo[s] * uo_sc[s, f]
                fb, uo = item
                ps_f = ps_fi.tile([33, 512], F32, tag="fi")
                nc.tensor.matmul(ps_f[0:1, :], wo_t[:], uo[:, 0:512])
                nc.tensor.matmul(ps_f[32:33, :], wo_t[:], uo[:, 512:1024])
                orow = orowpool.tile([1, F_IN], F32, tag="orow")
                nc.scalar.copy(orow[0:1, 0:512], ps_f[0:1, :])
                nc.scalar.copy(orow[0:1, 512:1024], ps_f[32:33, :])
                nc.sync.dma_start(out[fb : fb + 1, :], orow[:])

            for grp in range(n_groups):
                # ---- load x group: (128, KT, G, S) fp16 ----
                if grp == 0:
                    x16 = x16_first
                else:
                    x16 = xpool.tile([128, KT, G, S], F16, tag="x16")
                    nc.sync.dma_start(
                        x16[:], xr[grp].rearrange("p kt g s -> p (kt g s)")
                    )

                # ---- QK projections: q/k = W @ x (+bias), fp16 out ----
                q_sb = qkpool.tile([128, MT, NQK], F16, tag="q")
                k_sb = qkpool.tile([128, MT, NQK], F16, tag="k")
                for mt in range(MT):
                    for w_ts, b_t, dst in (
                        (wq_ts, bq_t, q_sb),
                        (wk_ts, bk_t, k_sb),
                    ):
                        ps = ps_qk.tile([128, NQK], F32, tag="qk")
                        for kt in range(KT):
                            nc.tensor.matmul(
                                ps[:],
                                w_ts[mt][:, kt, :],
                                x16[:, kt, :, :].rearrange("p g s -> p (g s)"),
                                start=(kt == 0),
                                stop=(kt == KT - 1),
                            )
                        # bias add + fp16 cast (DVE)
                        nc.vector.tensor_scalar_add(
                            dst[:, mt, :], ps[:], b_t[:, mt : mt + 1]
                        )

                # ---- attention per local batch ----
                for g in range(G):
                    b_loc = grp * G + g

                    # x[b]^T (with baked ones columns) straight from DRAM
                    xT = xtpool.tile([128, H, D + 1], F16, tag="xT")
                    nc.sync.dma_start(
                        xT[:], xtr[grp, :, g].rearrange("p h d -> p (h d)")
                    )

                    uo_sc = uopool.tile([128, H * D], F16, tag="uosc")
                    uo_view = uo_sc[:].rearrange(
                        "p (pair par d) -> p pair par d", par=2, d=D
                    )
                    # Octets: even heads -> cols 0-511 (bank A, PE rows 0-63),
                    # odd heads -> cols 512-1023 (bank B, rows 64-127).
                    # Matmuls into one PSUM bank must share a PE row group
                    # (mixing is concurrent and fatal), but across banks the
                    # two row groups run concurrently.
                    for oc in range(2):
                        ps_e = ps_sc.tile([128, 4 * S], F32, tag="sc")
                        ps_o = ps_sc.tile([128, 4 * S], F32, tag="sc")
                        for j in range(4):
                            mt = oc * 4 + j
                            for par, ps_s in ((0, ps_e), (1, ps_o)):
                                po = par * D
                                # scoresT[t, s] = sum_d k[d,t] * q[d,s]
                                nc.tensor.matmul(
                                    ps_s[:, j * S : (j + 1) * S],
                                    k_sb[po : po + D, mt, g * S : (g + 1) * S],
                                    q_sb[po : po + D, mt, g * S : (g + 1) * S],
                                )
                        for par, ps_s in ((0, ps_e), (1, ps_o)):
                            heads = [2 * (oc * 4 + j) + par for j in range(4)]
                            # one batched exp per parity quad (ACT)
                            E = epool.tile([128, 4 * S], F16, tag="E")
                            nc.scalar.activation(
                                E[:],
                                ps_s[:],
                                mybir.ActivationFunctionType.Exp,
                                bias=zero_t[:, 0:1],
                                scale=1.0 / TEMP,
                            )
                            ps_u = ps_uo.tile([128, 4, D + 1], F32, tag="uo")
                            for hi, h in enumerate(heads):
                                # uoutT[s, 0:64] = sum_t E[t,s] * xT[t, d]
                                # uoutT[s, 64]   = sum_t E[t,s]  (ones col)
                                nc.tensor.matmul(
                                    ps_u[:, hi, :],
                                    E[:, hi * S : (hi + 1) * S],
                                    xT[:, h, :],
                                )
                            # rc[s, hi] = 1 / colsum_hi[s]
                            rc = wfpool.tile([128, 4], F32, tag="rc")
                            nc.vector.reciprocal(rc[:], ps_u[:, :, D])
                            # uo_sc[s, (h,d)] = uoutT[s, (h,d)] * rc[s, h]
                            nc.vector.tensor_mul(
                                uo_view[:, oc * 4 : (oc + 1) * 4, par, :],
                                ps_u[:, :, 0:D],
                                rc[:].unsqueeze(2).broadcast_to((128, 4, D)),
                            )

                    # finals are deferred one batch so the PE never waits on
                    # the DVE normalization chain (software pipelining)
                    pending.append((b_loc, uo_sc))
                    if len(pending) > 1:
                        finalize(pending.pop(0))

            while pending:
                finalize(pending.pop(0))

    nc.compile()
    return nc


def prep_inputs(x, Wq, bq, Wk, bk, Wo, n_groups=8, G=4, n_cores=N_CORES):
    """Host-side shard + layout prep. Returns in_maps for run_bass_kernel_spmd."""
    x = np.asarray(x, dtype=np.float32)
    nb = n_groups * G
    x16 = x.astype(np.float16)
    # (c, grp, g, kt, p, s) -> (c, grp, p, kt, g, s)
    xr = (
        x16.reshape(n_cores, n_groups, G, KT, 128, S)
        .transpose(0, 1, 4, 3, 2, 5)
        .copy()
    )
    # x^T per batch with ones col per head: (c, grp, t, g, h, 65)
    xtr = np.ones((n_cores, n_groups, S, G, H, D + 1), dtype=np.float16)
    xtr[..., 0:D] = x16.reshape(n_cores, n_groups, G, H, D, S).transpose(
        0, 1, 5, 2, 3, 4
    )
    # W.T is (k, m); lay out as (mt, p, kt, 128) so each mt tile is one DMA
    wqt = np.ascontiguousarray(
        np.asarray(Wq, dtype=np.float32).T.reshape(KT, 128, MT, 128).transpose(2, 1, 0, 3)
    ).astype(np.float16)
    wkt = np.ascontiguousarray(
        np.asarray(Wk, dtype=np.float32).T.reshape(KT, 128, MT, 128).transpose(2, 1, 0, 3)
    ).astype(np.float16)
    bqr = np.ascontiguousarray(np.asarray(bq, dtype=np.float32).reshape(MT, 128).T)
    bkr = np.ascontiguousarray(np.asarray(bk, dtype=np.float32).reshape(MT, 128).T)
    wo_a = np.asarray(Wo, dtype=np.float32).reshape(128, 1).astype(np.float16)
    in_maps = []
    for c in range(n_cores):
        in_maps.append(
            {
                "xr": xr[c],
                "xtr": xtr[c],
                "wqt": wqt,
                "wkt": wkt,
                "bqr": bqr,
                "bkr": bkr,
                "wo16": wo_a,
            }
        )
    return in_maps


_NC_CACHE = {}


def kernel(x, Wq, bq, Wk, bk, Wo, bo):
    key = "full"
    if key not in _NC_CACHE:
        _NC_CACHE[key] = build_bass()
    nc = _NC_CACHE[key]

    in_maps = prep_inputs(x, Wq, bq, Wk, bk, Wo)
    res = run_bass_kernel_spmd(nc, in_maps, core_ids=list(range(N_CORES)), trace=TRACE)
    kernel.last_result = res
    out = np.concatenate([res.results[c]["out"] for c in range(N_CORES)], axis=0)
    out = out + np.float32(np.asarray(bo).reshape(-1)[0])
    return out.astype(np.float32)
